# revision 66
# baseline (speedup 1.0000x reference)
"""MoE GPT forward on 8 Trainium2 NeuronCores.

Sharding: batch-interleaved token-parallel residual stream (core c holds
tokens [128c,128c+128) of each batch), GRP8 RDH AllGather of local K/V for
attention (SBUF-resident consumption), expert-parallel MoE (1 expert per
core per layer) with token-major replicated routing (tri-matmul cumsum,
matmul-based slot->token inversion), column-parallel vocab head.

Precision: fp32r (11-bit RNE operand rounding, full-speed PE) on the
residual x-path, true fp32 for gate logits, fp32 accumulation everywhere.
"""
import sys
sys.path.insert(0, '/opt/trn_rl_repo')
from contextlib import ExitStack
import numpy as np
import ml_dtypes

V, S, H, NH, L, E, B = 32000, 1024, 768, 12, 2, 8, 2
DH = H // NH            # 64
FF = 4 * H              # 3072
T = B * S               # 2048
CAP = T // E            # 256
NCORE = 8
TL = T // NCORE         # 256 local tokens per core (128 per batch)
VS = V // NCORE         # 4000 vocab cols per core
HJ = H // 128           # 6
MFF = FF // 128         # 24
NV = 500
GRP8 = [[0, 1, 2, 3, 4, 5, 6, 7]]

_BUILT = {}


def _build(debug=False, stages=99):
    import concourse.bass as bass
    import concourse.mybir as mybir
    import concourse.tile as tile
    from concourse import bacc
    from concourse.bass import ts, ds
    from concourse.masks import make_identity

    f32 = mybir.dt.float32
    f32r = mybir.dt.float32r
    bf16 = mybir.dt.bfloat16
    i32 = mybir.dt.int32
    AF = mybir.ActivationFunctionType
    OP = mybir.AluOpType
    AX = mybir.AxisListType

    nc = bacc.Bacc("TRN2", target_bir_lowering=False, debug=False,
                   num_devices=NCORE)

    def din(name, shape, dt=f32):
        return nc.dram_tensor(name, shape, dt, kind="ExternalInput").ap()

    ids_l = din("ids_l", [TL, 1], i32)
    pos_l = din("pos_l", [TL, H])
    tokemb = din("tokemb", [V, H])
    tpos_l = din("tpos_l", [TL, 1], i32)
    starget = din("starget", [128, CAP])       # 256*c + s
    agrowf = din("agrowf", [128, 16])          # AG row per routing token
    qkT_w = [din(f"qkT_{l}", [12, 128, HJ, 128], f32r) for l in range(L)]
    vT_w = [din(f"vT_{l}", [H, H], f32r) for l in range(L)]
    qkb = [din(f"qkb_{l}", [128, 12]) for l in range(L)]
    vb_bc = [din(f"vb_{l}", [128, H]) for l in range(L)]
    outwT = [din(f"outwT_{l}", [H, H], f32r) for l in range(L)]
    outb_bc = [din(f"outb_{l}", [128, H]) for l in range(L)]
    ln1s = [din(f"ln1s_{l}", [128, H]) for l in range(L)]
    ln1b = [din(f"ln1b_{l}", [128, H]) for l in range(L)]
    ln2s = [din(f"ln2s_{l}", [128, H]) for l in range(L)]
    ln2b = [din(f"ln2b_{l}", [128, H]) for l in range(L)]
    gwT = [din(f"gwT_{l}", [H, E]) for l in range(L)]
    w1_w = [din(f"w1_{l}", [MFF, 128, HJ, 128], f32r) for l in range(L)]
    b1_w = [din(f"b1_{l}", [128, MFF]) for l in range(L)]
    w2_w = [din(f"w2_{l}", [FF, H], f32r) for l in range(L)]
    b2_bc = [din(f"b2_{l}", [128, H]) for l in range(L)]
    lnfs = din("lnfs", [128, H])
    lnfb = din("lnfb", [128, H])
    hwT = din("hwT", [VS // NV, 128, HJ, NV], bf16)

    out_l = nc.dram_tensor("out_l", [T, VS], f32, kind="ExternalOutput").ap()
    dbg = {}
    if debug:
        def dout(name, shape):
            dbg[name] = nc.dram_tensor("dbg_" + name, shape, f32,
                                       kind="ExternalOutput").ap()
        dout('xe', [TL, H])
        for l in range(L):
            dout(f'xa{l}', [TL, H])
            dout(f'x{l}', [TL, H])
            dout(f'lg{l}', [T, E])
            dout(f'rt{l}', [4, T])
            dout(f'sr{l}', [CAP, 1])
            dout(f'h2{l}', [CAP, H])

    with tile.TileContext(nc) as tc, ExitStack() as top:
        dram = top.enter_context(tc.tile_pool(name="dram", bufs=1, space="DRAM"))
        const = top.enter_context(tc.tile_pool(name="const", bufs=1))
        persist = top.enter_context(tc.tile_pool(name="persist", bufs=1))
        sb = top.enter_context(tc.tile_pool(name="sb", bufs=1))

        def dtile(name, shape, dt=f32, shared=False):
            return dram.tile(shape, dt, tag=name, name=name,
                             addr_space="Shared" if shared else "Local")

        kv_in = [dtile(f"kv_in{l}", [2 * H * TL], f32r) for l in range(L)]
        kv_out = [dtile(f"kv_out{l}", [NCORE * 2 * H * TL], f32r, True)
                  for l in range(L)]
        h2l_in = [dtile(f"h2l_in{l}", [TL, H + E]) for l in range(L)]
        h2l_out = [dtile(f"h2l_out{l}", [T, H + E], f32, True) for l in range(L)]
        dl_in = [dtile(f"dl_in{l}", [CAP, H]) for l in range(L)]
        dl_out = [dtile(f"dl_out{l}", [T, H], f32, True) for l in range(L)]
        hf_in = dtile("hf_in", [H, TL], bf16)
        hf_out = dtile("hf_out", [NCORE, H, TL], bf16, True)
        scr_srcgs = [dtile(f"scr_srcgs{l}", [T, 2]) for l in range(L)]

        # ---- constants ----
        ident = const.tile([128, 128], f32)
        make_identity(nc, ident)
        onesf = const.tile([128, 1], f32)
        nc.vector.memset(onesf[:], 1.0)
        iota8 = const.tile([128, 8], i32)
        nc.gpsimd.iota(iota8[:], [[1, 8]], channel_multiplier=0)
        iota8f = const.tile([128, 8], f32)
        nc.vector.tensor_copy(iota8f[:], iota8[:])
        iota_p = const.tile([128, 1], i32)
        nc.gpsimd.iota(iota_p[:], [[0, 1]], channel_multiplier=1)
        iota_row = const.tile([128, 128], i32)
        nc.gpsimd.iota(iota_row[:], [[1, 128]], channel_multiplier=0)
        # strict lower-triangular ones: tri[p, m] = 1.0 if m > p
        iota_pf = const.tile([128, 1], f32)
        nc.vector.tensor_copy(iota_pf[:], iota_p[:])
        iota_rowf = const.tile([128, 128], f32)
        nc.vector.tensor_copy(iota_rowf[:], iota_row[:])
        tri_s = const.tile([128, 128], f32)
        nc.vector.tensor_scalar(tri_s[:], iota_rowf[:], iota_pf[:, :1], None,
                                op0=OP.is_gt)
        eps_t = const.tile([128, 1], f32)
        nc.vector.memset(eps_t[:], 1e-5)
        ones64r = const.tile([128, 64], f32r)
        nc.vector.tensor_copy(ones64r[:], onesf[:].to_broadcast([128, 64]))
        agrow_sb = const.tile([128, 16], f32)
        nc.sync.dma_start(agrow_sb[:], agrowf)
        starg_sb = const.tile([128, CAP], f32)
        nc.sync.dma_start(starg_sb[:], starget)

        x_sb = persist.tile([128, 2, H], f32, tag="x_sb")

        # ================= embedding =================
        for k in range(2):
            idt = sb.tile([128, 1], i32, tag="idt", bufs=2)
            nc.sync.dma_start(idt[:], ids_l[ds(128 * k, 128), :])
            emb = sb.tile([128, H], f32, tag="emb", bufs=2)
            nc.gpsimd.indirect_dma_start(
                out=emb[:], out_offset=None, in_=tokemb,
                in_offset=bass.IndirectOffsetOnAxis(ap=idt[:, :1], axis=0))
            post = sb.tile([128, H], f32, tag="post", bufs=2)
            nc.sync.dma_start(post[:], pos_l[ds(128 * k, 128), :])
            nc.vector.tensor_add(x_sb[:, k, :], emb[:], post[:])
        if debug:
            nc.sync.dma_start(dbg['xe'].rearrange("(k p) d -> p k d", p=128), x_sb[:])

        def layer_norm(dst, src_view, s_bc_d, b_bc_d):
            s_bc = sb.tile([128, H], f32, tag="ln_s")
            b_bc = sb.tile([128, H], f32, tag="ln_b")
            nc.sync.dma_start(s_bc[:], s_bc_d)
            nc.sync.dma_start(b_bc[:], b_bc_d)
            for k in range(2):
                mean = sb.tile([128, 1], f32, tag="ln_m", bufs=2)
                nc.vector.tensor_reduce(mean[:], src_view[:, k, :], axis=AX.X,
                                        op=OP.add)
                nc.vector.tensor_scalar_mul(mean[:], mean[:], 1.0 / H)
                xm = sb.tile([128, H], f32, tag="ln_xm", bufs=2)
                nc.vector.tensor_scalar_sub(xm[:], src_view[:, k, :], mean[:, :1])
                sq = sb.tile([128, H], f32, tag="ln_sq", bufs=2)
                nc.vector.tensor_tensor(sq[:], xm[:], xm[:], op=OP.mult)
                var = sb.tile([128, 1], f32, tag="ln_v", bufs=2)
                nc.vector.tensor_reduce(var[:], sq[:], axis=AX.X, op=OP.add)
                nc.vector.tensor_scalar_mul(var[:], var[:], 1.0 / H)
                sd = sb.tile([128, 1], f32, tag="ln_sd", bufs=2)
                nc.scalar.activation(sd[:], var[:], AF.Sqrt, bias=eps_t[:, :1])
                rstd = sb.tile([128, 1], f32, tag="ln_r", bufs=2)
                nc.vector.reciprocal(rstd[:], sd[:])
                nc.vector.scalar_tensor_tensor(
                    out=dst[:, k, :], in0=xm[:], scalar=rstd[:, :1], in1=s_bc[:],
                    op0=OP.mult, op1=OP.mult)
                nc.vector.tensor_add(dst[:, k, :], dst[:, k, :], b_bc[:])

        def transpose_2H(src_view, dst):
            """src [128,2,H] fp32 token-major -> dst [128, HJ, TL] feature-major."""
            with tc.tile_pool(name="pst", bufs=4, space="PSUM") as pst:
                for k in range(2):
                    for j in range(HJ):
                        pt = pst.tile([128, 128], f32, tag="pt", bufs=4)
                        nc.tensor.transpose(pt[:], src_view[:, k, ts(j, 128)],
                                            ident[:])
                        nc.vector.tensor_copy(dst[:, j, ds(128 * k, 128)], pt[:])

        # ================= layers =================
        nlayers = 0 if stages <= 1 else (1 if stages < 6 else L)
        for l in range(nlayers):
            with ExitStack() as lyr:
                abuf_cm = tc.tile_pool(name="abuf", bufs=1)
                abuf = abuf_cm.__enter__()
                aqk_cm = tc.tile_pool(name="aqk", bufs=1)
                aqk = aqk_cm.__enter__()
                # prefetch this layer's attention weights first (overlaps the
                # previous layer's dl allgather via sync-queue order)
                wqk_all = aqk.tile([128, 12, HJ, 128], f32r, tag="wqk_all")
                for m in range(12):
                    nc.sync.dma_start(wqk_all[:, m, :, :], qkT_w[l][m])
                wv = aqk.tile([128, HJ, H], f32r, tag="wv")
                for j in range(HJ):
                    nc.sync.dma_start(wv[:, j, :], vT_w[l][ds(128 * j, 128), :])
                qkbias = sb.tile([128, 12], f32, tag="qkb")
                nc.sync.dma_start(qkbias[:], qkb[l])
                vbias = sb.tile([128, H], f32, tag="vbias")
                nc.sync.dma_start(vbias[:], vb_bc[l])
                h1 = aqk.tile([128, 2, H], f32, tag="h1")
                layer_norm(h1, x_sb, ln1s[l], ln1b[l])
                h1T = aqk.tile([128, HJ, TL], f32r, tag="h1T")
                transpose_2H(h1, h1T)

                # ---- QKV (local tokens) ----
                qkT = abuf.tile([128, 12, TL], f32r, tag="qkT")
                v_sb = aqk.tile([128, 2, H], f32r, tag="v_sb")
                with tc.tile_pool(name="psq", bufs=2, space="PSUM") as psq:
                    for m in range(12):
                        pq = psq.tile([128, TL], f32, tag="pq", bufs=2)
                        for j in range(HJ):
                            nc.tensor.matmul(pq[:], wqk_all[:, m, j, :],
                                             h1T[:, j, :],
                                             start=(j == 0), stop=(j == HJ - 1))
                        nc.vector.tensor_scalar_add(qkT[:, m, :], pq[:],
                                                    qkbias[:, m:m + 1])
                    for k in range(2):
                        for nn in range(2):
                            pv = psq.tile([128, 384], f32, tag="pv", bufs=2)
                            for j in range(HJ):
                                nc.tensor.matmul(pv[:], h1T[:, j, ts(k, 128)],
                                                 wv[:, j, ds(384 * nn, 384)],
                                                 start=(j == 0), stop=(j == HJ - 1))
                            nc.vector.tensor_add(v_sb[:, k, ds(384 * nn, 384)],
                                                 pv[:], vbias[:, ds(384 * nn, 384)])

                # ---- kv allgather (all 8 cores) ----
                kT_view = kv_in[l][:H * TL].rearrange("(m p t) -> p m t", p=128, t=TL)
                v_view = kv_in[l][H * TL:].rearrange("(k p d) -> p k d", p=128, d=H)
                nc.sync.dma_start(kT_view, qkT[:, 6:12, :])
                nc.sync.dma_start(v_view, v_sb[:])
                aqk_cm.__exit__(None, None, None)
                nc.gpsimd.collective_compute(
                    "AllGather", OP.bypass, replica_groups=GRP8,
                    ins=[kv_in[l][:]], outs=[kv_out[l][:]])

                # ---- attention ----
                do_attn = stages >= 3
                ctx64 = abuf.tile([64, NH, TL], f32r, tag="ctx64")
                if not do_attn:
                    zct = sb.tile([64, NH, TL], f32, tag="zct")
                    nc.vector.memset(zct[:], 0.0)
                    nc.vector.tensor_copy(ctx64[:], zct[:])
                with tc.tile_pool(name="psa", bufs=4, space="PSUM") as psa, \
                     tc.tile_pool(name="ab2", bufs=2) as ab2, \
                     tc.tile_pool(name="abk", bufs=1) as abk:
                  if do_attn:
                    # load gathered kT/v into SBUF (contiguous slabs)
                    # kTg[p, j, 256*r + kcol]: feature 128*j+p, key slab r
                    # vg[p(key within chunk), ck=2*r+k, d]: key chunk of 128
                    kTg = abk.tile([128, HJ, T], f32r, tag="kTg", name="kTg")
                    vg = abk.tile([128, 2 * NCORE, H], f32r, tag="vg", name="vg")
                    for r in range(NCORE):
                        koff = r * (2 * H * TL)
                        kT_c = kv_out[l][ds(koff, H * TL)].rearrange(
                            "(m p t) -> p m t", p=128, t=TL)
                        v_c = kv_out[l][ds(koff + H * TL, H * TL)].rearrange(
                            "(k p d) -> p k d", p=128, d=H)
                        nc.sync.dma_start(kTg[:, :, ds(TL * r, TL)], kT_c)
                        nc.sync.dma_start(vg[:, ds(2 * r, 2), :], v_c)
                    # per (head, query-half): keys of matching batch
                    # key chunk ck for batch k: ck = 2*r + k, r = 0..7
                    for h in range(NH):
                        po = 64 * (h % 2)
                        jq = h // 2
                        for k in range(2):
                            pss = psa.tile([128, NCORE, 128], f32, tag="pss",
                                           bufs=2)
                            for r in range(NCORE):
                                nc.tensor.matmul(
                                    pss[:, r, :],
                                    kTg[ds(po, 64), jq, ds(256 * r + 128 * k, 128)],
                                    qkT[ds(po, 64), jq, ds(128 * k, 128)],
                                    start=True, stop=True)
                            expT = ab2.tile([128, NCORE, 128], f32r, tag="expT",
                                            bufs=3)
                            nc.scalar.activation(expT[:], pss[:], AF.Exp,
                                                 scale=1.0 / np.sqrt(DH))
                            psum_s = psa.tile([64, 128], f32, tag="psum_s", bufs=2)
                            for r in range(NCORE):
                                nc.tensor.matmul(psum_s[:], ones64r[:],
                                                 expT[:, r, :],
                                                 start=(r == 0), stop=(r == NCORE - 1))
                            rbc = ab2.tile([64, 128], f32, tag="rbc", bufs=2)
                            nc.vector.reciprocal(rbc[:], psum_s[:])
                            pc = psa.tile([64, 128], f32, tag="pc", bufs=2)
                            for r in range(NCORE):
                                nc.tensor.matmul(
                                    pc[:], vg[:, 2 * r + k, ds(64 * h, 64)],
                                    expT[:, r, :],
                                    start=(r == 0), stop=(r == NCORE - 1))
                            nc.vector.tensor_tensor(
                                ctx64[:, h, ds(128 * k, 128)], pc[:],
                                rbc[:], op=OP.mult)

                # ---- out-proj + residual ----
                with tc.tile_pool(name="pso", bufs=2, space="PSUM") as pso, \
                     tc.tile_pool(name="wo_p", bufs=1) as wo_p:
                    wo64 = wo_p.tile([64, NH, H], f32r, tag="wo64")
                    for h in range(NH):
                        nc.sync.dma_start(wo64[:, h, :], outwT[l][ds(64 * h, 64), :])
                    obias = sb.tile([128, H], f32, tag="obias")
                    nc.sync.dma_start(obias[:], outb_bc[l])
                    for k in range(2):
                        for nn in range(2):
                            pol = pso.tile([128, 384], f32, tag="pol", bufs=2)
                            for h in range(NH):
                                nc.tensor.matmul(pol[:], ctx64[:, h, ts(k, 128)],
                                                 wo64[:, h, ds(384 * nn, 384)],
                                                 start=(h == 0), stop=(h == NH - 1))
                            sl = ds(384 * nn, 384)
                            nc.vector.tensor_add(x_sb[:, k, sl], x_sb[:, k, sl],
                                                 pol[:])
                        nc.vector.tensor_add(x_sb[:, k, :], x_sb[:, k, :], obias[:])
                if debug:
                    nc.sync.dma_start(dbg[f'xa{l}'].rearrange("(k p) d -> p k d", p=128), x_sb[:])

                abuf_cm.__exit__(None, None, None)
                if stages < 4:
                    continue
                # ---- LN2 + gate logits ----
                mbuf = lyr.enter_context(tc.tile_pool(name="mbuf", bufs=1))
                h2 = mbuf.tile([128, 2, H], f32, tag="h2")
                layer_norm(h2, x_sb, ln2s[l], ln2b[l])
                h2T = mbuf.tile([128, HJ, TL], f32, tag="h2T")
                transpose_2H(h2, h2T)
                lg_loc = sb.tile([128, 2, E], f32, tag="lg_loc")
                with tc.tile_pool(name="psg", bufs=2, space="PSUM") as psg:
                    gw = sb.tile([128, HJ, E], f32, tag="gw")
                    for j in range(HJ):
                        nc.sync.dma_start(gw[:, j, :], gwT[l][ds(128 * j, 128), :])
                    for k in range(2):
                        pg = psg.tile([128, E], f32, tag="pg", bufs=2)
                        for j in range(HJ):
                            nc.tensor.matmul(pg[:], h2T[:, j, ts(k, 128)],
                                             gw[:, j, :],
                                             start=(j == 0), stop=(j == HJ - 1))
                        nc.vector.tensor_copy(lg_loc[:, k, :], pg[:])
                # single AG payload: [h2 | logits] per token row
                nc.sync.dma_start(
                    h2l_in[l][:, :H].rearrange("(k p) d -> p k d", p=128), h2[:])
                nc.sync.dma_start(
                    h2l_in[l][:, H:].rearrange("(k p) e -> p k e", p=128),
                    lg_loc[:])
                nc.gpsimd.collective_compute(
                    "AllGather", OP.bypass, replica_groups=GRP8,
                    ins=[h2l_in[l][:].rearrange("a b -> (a b)")],
                    outs=[h2l_out[l][:].rearrange("a b -> (a b)")])
                # prefetch full w1 while the collectives + routing run
                w1p_cm = tc.tile_pool(name="w1p", bufs=1)
                w1p = w1p_cm.__enter__()
                w1sb = w1p.tile([128, MFF, HJ, 128], f32r, tag="w1sb")
                for m in range(MFF):
                    nc.sync.dma_start(w1sb[:, m, :, :], w1_w[l][m])
                b1s = sb.tile([128, MFF], f32, tag="b1s")
                nc.sync.dma_start(b1s[:], b1_w[l])
                b2s = sb.tile([128, H], f32, tag="b2s")
                nc.sync.dma_start(b2s[:], b2_bc[l])

                # ---- routing (token-major, replicated on all cores) ----
                # routing layout [p, q]: token g = 1024*(p//64) + 128*((p%64)//8)
                #                                + 16*(p%8) + q  (lexicographic)
                srci2 = sb.tile([128, 2], i32, tag="srci2")
                with tc.tile_pool(name="rt", bufs=1) as rt, \
                     tc.tile_pool(name="psr", bufs=4, space="PSUM") as psr:
                    lg = rt.tile([128, 16, E], f32, tag="lg")
                    for bb in range(2):
                        for r in range(8):
                            nc.sync.dma_start(
                                lg[ds(64 * bb + 8 * r, 8), :, :],
                                h2l_out[l][ds(256 * r + 128 * bb, 128), H:]
                                .rearrange("(p2 q) e -> p2 q e", p2=8))
                    if debug:
                        for bb in range(2):
                            for r in range(8):
                                nc.sync.dma_start(
                                    dbg[f'lg{l}'][ds(1024 * bb + 128 * r, 128), :]
                                    .rearrange("(p2 q) e -> p2 (q e)", p2=8),
                                    lg[ds(64 * bb + 8 * r, 8), :, :])
                    ex = rt.tile([128, 16, E], f32, tag="ex")
                    nc.scalar.activation(ex[:], lg[:], AF.Exp)
                    mx = rt.tile([128, 16, 1], f32, tag="mx")
                    nc.vector.tensor_reduce(mx[:], ex[:], axis=AX.X, op=OP.max)
                    sm = rt.tile([128, 16, 1], f32, tag="sm")
                    nc.vector.tensor_reduce(sm[:], ex[:], axis=AX.X, op=OP.add)
                    rsm = rt.tile([128, 16, 1], f32, tag="rsm")
                    nc.vector.reciprocal(rsm[:], sm[:])
                    gp = rt.tile([128, 16], f32, tag="gp")
                    nc.vector.tensor_tensor(gp[:], mx[:, :, 0], rsm[:, :, 0],
                                            op=OP.mult)
                    mask = rt.tile([128, 16, E], f32, tag="mask")
                    nc.vector.tensor_tensor(mask[:], ex[:],
                                            mx[:].to_broadcast([128, 16, E]),
                                            op=OP.is_equal)
                    eqi = rt.tile([128, 16, E], f32, tag="eqi")
                    nc.vector.tensor_tensor(
                        eqi[:], mask[:],
                        iota8f[:, None, :].to_broadcast([128, 16, E]), op=OP.mult)
                    idxf = rt.tile([128, 16, 1], f32, tag="idxf")
                    nc.vector.tensor_reduce(idxf[:], eqi[:], axis=AX.X, op=OP.add)
                    # inclusive cumsum over q (shift-adds), then partition prefix
                    c0 = rt.tile([128, 16, E], f32, tag="c0")
                    c1 = rt.tile([128, 16, E], f32, tag="c1")
                    nc.vector.tensor_copy(c0[:], mask[:])
                    for s, (a, b) in zip((1, 2, 4, 8),
                                         ((c0, c1), (c1, c0), (c0, c1), (c1, c0))):
                        nc.vector.tensor_copy(b[:, :s, :], a[:, :s, :])
                        nc.vector.tensor_add(b[:, s:, :], a[:, s:, :],
                                             a[:, :16 - s, :])
                    # c0 holds the q-inclusive cumsum
                    ppref = psr.tile([128, E], f32, tag="ppref", bufs=2)
                    nc.tensor.matmul(ppref[:], tri_s[:], c0[:, 15, :],
                                     start=True, stop=True)
                    pprefs = rt.tile([128, 1, E], f32, tag="pprefs")
                    nc.vector.tensor_copy(pprefs[:, 0, :], ppref[:])
                    locs = rt.tile([128, 16, E], f32, tag="locs")
                    nc.vector.tensor_tensor(
                        locs[:], c0[:], pprefs[:].to_broadcast([128, 16, E]),
                        op=OP.add)
                    kept = rt.tile([128, 16, E], f32, tag="kept")
                    nc.vector.tensor_scalar(kept[:], locs[:], float(CAP), None,
                                            op0=OP.is_le)
                    nc.vector.tensor_tensor(kept[:], kept[:], mask[:], op=OP.mult)
                    keptany = rt.tile([128, 16, 1], f32, tag="keptany")
                    nc.vector.tensor_reduce(keptany[:], kept[:], axis=AX.X,
                                            op=OP.add)
                    lockept = rt.tile([128, 16, E], f32, tag="lockept")
                    nc.vector.tensor_tensor(lockept[:], locs[:], kept[:],
                                            op=OP.mult)
                    loc1 = rt.tile([128, 16, 1], f32, tag="loc1")
                    nc.vector.tensor_reduce(loc1[:], lockept[:], axis=AX.X,
                                            op=OP.add)
                    # slot id: a_raw = CAP*idx + loc1 - 1 (unmasked)
                    a_raw = rt.tile([128, 16], f32, tag="a_raw")
                    nc.vector.scalar_tensor_tensor(
                        out=a_raw[:], in0=idxf[:, :, 0], scalar=float(CAP),
                        in1=loc1[:, :, 0], op0=OP.mult, op1=OP.add)
                    nc.vector.tensor_scalar_add(a_raw[:], a_raw[:], -1.0)
                    # combine scratch: src = a_raw*kept, gs = gp*kept
                    sg = rt.tile([128, 16, 2], f32, tag="sg")
                    nc.vector.tensor_tensor(sg[:, :, 0], a_raw[:],
                                            keptany[:, :, 0], op=OP.mult)
                    nc.vector.tensor_tensor(sg[:, :, 1], gp[:],
                                            keptany[:, :, 0], op=OP.mult)
                    for bb in range(2):
                        for r in range(8):
                            nc.sync.dma_start(
                                scr_srcgs[l][ds(1024 * bb + 128 * r, 128), :]
                                .rearrange("(p2 q) s -> p2 (q s)", p2=8),
                                sg[ds(64 * bb + 8 * r, 8), :, :])
                    if debug:
                        # raw [p, q] layout; numpy side maps t = p*16+q -> g
                        nc.sync.dma_start(
                            dbg[f'rt{l}'][0, :].rearrange("(p q) -> p q", p=128),
                            idxf[:, :, 0])
                        nc.sync.dma_start(
                            dbg[f'rt{l}'][1, :].rearrange("(p q) -> p q", p=128),
                            sg[:, :, 0])
                        nc.sync.dma_start(
                            dbg[f'rt{l}'][2, :].rearrange("(p q) -> p q", p=128),
                            sg[:, :, 1])
                        nc.sync.dma_start(
                            dbg[f'rt{l}'][3, :].rearrange("(p q) -> p q", p=128),
                            keptany[:, :, 0])
                    # slot -> src AG row inversion for OUR expert
                    # slot_val = a_raw + 1e6*(1-keptany): dropped never match
                    sval = rt.tile([128, 16], f32, tag="sval")
                    nc.vector.tensor_scalar(sval[:], keptany[:, :, 0], -1e6, 1e6,
                                            op0=OP.mult, op1=OP.add)
                    nc.vector.tensor_add(sval[:], sval[:], a_raw[:])
                    pinv = [psr.tile([128, 1], f32, tag=f"pinv{sc}",
                                     name=f"pinv{sc}_{l}", bufs=1)
                            for sc in range(2)]
                    for q in range(16):
                        ind_q = rt.tile([128, CAP], f32, tag="ind_q", bufs=3)
                        nc.vector.tensor_scalar(ind_q[:], starg_sb[:],
                                                sval[:, q:q + 1], None,
                                                op0=OP.is_equal)
                        for sc in range(2):
                            nc.tensor.matmul(pinv[sc][:],
                                             ind_q[:, ds(128 * sc, 128)],
                                             agrow_sb[:, q:q + 1],
                                             start=(q == 0), stop=(q == 15))
                    srcf = sb.tile([128, 2], f32, tag="srcf")
                    for sc in range(2):
                        nc.vector.tensor_copy(srcf[:, sc:sc + 1], pinv[sc][:])
                    nc.vector.tensor_copy(srci2[:], srcf[:])
                    if debug:
                        nc.sync.dma_start(
                            dbg[f'sr{l}'].rearrange("(sc p) one -> p (sc one)",
                                                    p=128), srcf[:])

                if stages < 5:
                    continue
                # ---- MoE FFN (this core's expert) ----
                xsT = mbuf.tile([128, HJ, CAP], f32r, tag="xsT")
                with tc.tile_pool(name="psx", bufs=4, space="PSUM") as psx, \
                     tc.tile_pool(name="mb", bufs=2) as mb:
                    for k in range(2):
                        xg = mb.tile([128, H + E], f32, tag="xg", bufs=2)
                        nc.gpsimd.indirect_dma_start(
                            out=xg[:], out_offset=None, in_=h2l_out[l][:],
                            in_offset=bass.IndirectOffsetOnAxis(
                                ap=srci2[:, k:k + 1], axis=0))
                        for j in range(HJ):
                            pt = psx.tile([128, 128], f32, tag="xtp", bufs=4)
                            nc.tensor.transpose(pt[:], xg[:, ts(j, 128)], ident[:])
                            nc.vector.tensor_copy(xsT[:, j, ds(128 * k, 128)],
                                                  pt[:])
                h1T_m = mbuf.tile([128, MFF, CAP], f32r, tag="h1T_m")
                with tc.tile_pool(name="psm", bufs=2, space="PSUM") as psm:
                    for m in range(MFF):
                        ph = psm.tile([128, CAP], f32, tag="ph", bufs=2)
                        for j in range(HJ):
                            nc.tensor.matmul(ph[:], w1sb[:, m, j, :], xsT[:, j, :],
                                             start=(j == 0), stop=(j == HJ - 1))
                        nc.scalar.activation(h1T_m[:, m, :], ph[:], AF.Gelu,
                                             bias=b1s[:, m:m + 1])
                w1p_cm.__exit__(None, None, None)
                dsb = sb.tile([128, 2, H], f32, tag="dsb")
                with tc.tile_pool(name="psd", bufs=1, space="PSUM") as psd, \
                     tc.tile_pool(name="wst2", bufs=12) as wst2:
                    pd = [[psd.tile([128, 384], f32, tag=f"pd{k}{nn}", name=f"pd{k}{nn}_{l}", bufs=1)
                           for nn in range(2)] for k in range(2)]
                    for m in range(MFF):
                        w2c = wst2.tile([128, 1, H], f32r, tag="w2c", bufs=12)
                        nc.sync.dma_start(w2c[:, 0, :], w2_w[l][ds(128 * m, 128), :])
                        for k in range(2):
                            for nn in range(2):
                                nc.tensor.matmul(pd[k][nn][:],
                                                 h1T_m[:, m, ts(k, 128)],
                                                 w2c[:, 0, ds(384 * nn, 384)],
                                                 start=(m == 0),
                                                 stop=(m == MFF - 1))
                    for k in range(2):
                        for nn in range(2):
                            sl = ds(384 * nn, 384)
                            nc.vector.tensor_add(dsb[:, k, sl], pd[k][nn][:],
                                                 b2s[:, sl])
                if debug:
                    nc.sync.dma_start(dbg[f'h2{l}'].rearrange("(k p) d -> p k d", p=128), dsb[:])
                nc.sync.dma_start(
                    dl_in[l][:].rearrange("(k p) d -> p k d", p=128), dsb[:])
                nc.gpsimd.collective_compute(
                    "AllGather", OP.bypass, replica_groups=GRP8,
                    ins=[dl_in[l][:].rearrange("a b -> (a b)")],
                    outs=[dl_out[l][:].rearrange("a b -> (a b)")])

                # ---- combine ----
                for k in range(2):
                    tp = sb.tile([128, 1], i32, tag="tp", bufs=2)
                    nc.sync.dma_start(tp[:], tpos_l[ds(128 * k, 128), :])
                    sgf = sb.tile([128, 2], f32, tag="sgf", bufs=2)
                    nc.gpsimd.indirect_dma_start(
                        out=sgf[:], out_offset=None, in_=scr_srcgs[l][:],
                        in_offset=bass.IndirectOffsetOnAxis(ap=tp[:, :1], axis=0))
                    srci = sb.tile([128, 1], i32, tag="srci", bufs=2)
                    nc.vector.tensor_copy(srci[:], sgf[:, 0:1])
                    dg = sb.tile([128, H], f32, tag="dg", bufs=2)
                    nc.gpsimd.indirect_dma_start(
                        out=dg[:], out_offset=None, in_=dl_out[l][:],
                        in_offset=bass.IndirectOffsetOnAxis(ap=srci[:, :1], axis=0))
                    nc.vector.scalar_tensor_tensor(
                        out=x_sb[:, k, :], in0=dg[:], scalar=sgf[:, 1:2],
                        in1=x_sb[:, k, :], op0=OP.mult, op1=OP.add)
                if debug:
                    nc.sync.dma_start(dbg[f'x{l}'].rearrange("(k p) d -> p k d", p=128), x_sb[:])

        # ================= final LN + head =================
        with ExitStack() as fin:
          if stages >= 7:
            fb = fin.enter_context(tc.tile_pool(name="fb", bufs=1))
            # preload all head-weight chunks (bf16, 48KB) — overlaps the last
            # dl allgather + combine + final LN via sync-queue order
            hw_all = fb.tile([128, VS // NV, HJ, NV], bf16, tag="hw_all")
            for n in range(VS // NV):
                nc.sync.dma_start(hw_all[:, n, :, :], hwT[n])
            hf = fb.tile([128, 2, H], f32, tag="hf")
            layer_norm(hf, x_sb, lnfs, lnfb)
            hfT = fb.tile([128, HJ, TL], bf16, tag="hfT")
            transpose_2H(hf, hfT)
            nc.sync.dma_start(
                hf_in[:].rearrange("(j p) t -> p j t", p=128), hfT[:])
            nc.gpsimd.collective_compute(
                "AllGather", OP.bypass, replica_groups=GRP8,
                ins=[hf_in[:]], outs=[hf_out[:]])
            hfT_all = fb.tile([128, HJ, T], bf16, tag="hfT_all")
            for r in range(NCORE):
                nc.sync.dma_start(
                    hfT_all[:, :, ds(TL * r, TL)],
                    hf_out[r].rearrange("(j p) t -> p j t", p=128))
            psh = fin.enter_context(tc.tile_pool(name="psh", bufs=2, space="PSUM"))
            hwp = fin.enter_context(tc.tile_pool(name="hwp", bufs=2))
            for t_ in range(T // 128):
                # AG col chunk t_ -> global token rows; t_-major so the first
                # matmuls only wait on slab t_//2 of the hfT_all loads
                tg = 1024 * (t_ % 2) + 128 * (t_ // 2)
                for n in range(VS // NV):
                    po_ = psh.tile([128, NV], f32, tag="po_", bufs=4)
                    for j in range(HJ):
                        nc.tensor.matmul(po_[:], hfT_all[:, j, ts(t_, 128)],
                                         hw_all[:, n, j, :],
                                         start=(j == 0), stop=(j == HJ - 1))
                    osb = hwp.tile([128, NV], f32, tag="osb", bufs=4)
                    if n % 2 == 0:
                        nc.vector.tensor_copy(osb[:], po_[:])
                    else:
                        nc.scalar.copy(osb[:], po_[:])
                    nc.sync.dma_start(out_l[ds(tg, 128), ds(NV * n, NV)],
                                      osb[:])

    nc.compile()
    return nc


def _shard_inputs(inputs):
    f = lambda a: np.ascontiguousarray(np.asarray(a), dtype=np.float32)
    ids = np.asarray(inputs['input_ids']).astype(np.int32)  # [B, S]
    tokemb = f(inputs['token_emb'])
    pos = f(inputs['pos_emb'])
    bc = lambda vv: np.tile(f(vv).reshape(1, H), (128, 1))
    # routing token (p, q): g = 1024*(p//64) + 128*((p%64)//8) + 16*(p%8) + q
    # AG row of token g: 256*((g%1024)//128) + 128*(g//1024) + (g%128)
    pp = np.arange(128)
    qq = np.arange(16)
    gg = (1024 * (pp[:, None] // 64) + 128 * ((pp[:, None] % 64) // 8)
          + 16 * (pp[:, None] % 8) + qq[None, :])          # [128,16] global id
    agrow = (256 * ((gg % 1024) // 128) + 128 * (gg // 1024)
             + (gg % 128)).astype(np.float32)
    in_maps = []
    for c in range(NCORE):
        sl = slice(128 * c, 128 * (c + 1))
        m = {
            'ids_l': np.concatenate([ids[0, sl], ids[1, sl]]).reshape(TL, 1),
            'pos_l': np.ascontiguousarray(
                np.concatenate([pos[sl], pos[sl]], axis=0)),
            'tokemb': tokemb,
            'tpos_l': np.concatenate(
                [np.arange(128 * c, 128 * (c + 1)),
                 np.arange(S + 128 * c, S + 128 * (c + 1))]
            ).astype(np.int32).reshape(TL, 1),
            'starget': np.tile(
                (CAP * c + np.arange(CAP, dtype=np.float32))[None, :],
                (128, 1)),
            'agrowf': agrow,
            'lnfs': bc(inputs['lnf_scale']), 'lnfb': bc(inputs['lnf_bias']),
            # head_w slice transposed [H, VS] -> packed [VS//NV, 128, HJ, NV]
            'hwT': np.ascontiguousarray(
                f(inputs['head_w'])[VS * c:VS * (c + 1)].T
                .reshape(HJ, 128, VS // NV, NV).transpose(2, 1, 0, 3)
                .astype(ml_dtypes.bfloat16)),
        }
        for l in range(L):
            in_w = f(inputs['attn_in_w'][l])
            in_b = f(inputs['attn_in_b'][l])
            # [H, 2H] -> packed [12, 128, HJ, 128]
            m[f'qkT_{l}'] = np.ascontiguousarray(
                in_w[:2 * H].T.reshape(HJ, 128, 12, 128).transpose(2, 1, 0, 3))
            m[f'vT_{l}'] = np.ascontiguousarray(in_w[2 * H:].T)
            m[f'qkb_{l}'] = np.ascontiguousarray(in_b[:2 * H].reshape(12, 128).T)
            m[f'vb_{l}'] = np.tile(in_b[2 * H:].reshape(1, H), (128, 1))
            m[f'outwT_{l}'] = np.ascontiguousarray(f(inputs['attn_out_w'][l]).T)
            m[f'outb_{l}'] = bc(inputs['attn_out_b'][l])
            m[f'ln1s_{l}'] = bc(inputs['ln1_scale'][l])
            m[f'ln1b_{l}'] = bc(inputs['ln1_bias'][l])
            m[f'ln2s_{l}'] = bc(inputs['ln2_scale'][l])
            m[f'ln2b_{l}'] = bc(inputs['ln2_bias'][l])
            m[f'gwT_{l}'] = np.ascontiguousarray(f(inputs['gate_w'][l]).T)
            # [H, FF] -> packed [MFF, 128, HJ, 128]
            m[f'w1_{l}'] = np.ascontiguousarray(
                f(inputs['w1'][l, c]).reshape(HJ, 128, MFF, 128)
                .transpose(2, 1, 0, 3))
            m[f'b1_{l}'] = np.ascontiguousarray(
                f(inputs['b1'][l, c]).reshape(MFF, 128).T)
            m[f'w2_{l}'] = f(inputs['w2'][l, c])
            m[f'b2_{l}'] = np.tile(f(inputs['b2'][l, c]).reshape(1, H), (128, 1))
        in_maps.append(m)
    return in_maps


def run(inputs, debug=False, trace=False, stages=99, **kw):
    from concourse.bass_utils import run_bass_kernel_spmd
    key = (bool(debug), stages)
    if key not in _BUILT:
        _BUILT[key] = _build(debug=debug, stages=stages)
    nc = _BUILT[key]
    in_maps = _shard_inputs(inputs)
    return run_bass_kernel_spmd(nc, in_maps, core_ids=list(range(NCORE)),
                                trace=trace, **kw)


def kernel(**inputs):
    res = run(inputs, debug=False)
    out = np.concatenate([res.results[c]['out_l'] for c in range(NCORE)], axis=1)
    return out.reshape(B, S, V)
